# revision 30
# baseline (speedup 1.0000x reference)
"""Trainium2 Bass kernel for an ensemble Q-network MLP.

Reference computation (per ensemble member e):
    h1 = relu(x @ W1[e] + b1[e])      x: [B, IN]
    h2 = relu(h1 @ W2[e] + b2[e])
    out[:, e, :] = h2 @ W3[e] + b3[e]  -> [B, E, OUT]

Shapes: B=16384, IN=256, H=512, OUT=32, E=8, fp32.

Sharding across 8 NeuronCores: 2 ensemble groups x 4 batch shards.
Each core processes EG=4 members on BC=4096 batch rows (equal FLOPs per
core; minimizes total DMA volume per core).

On-core plan (activations kept feature-major [feature_part, batch_free] so
the contraction dim always lands on SBUF partitions):
  - Weights DMA straight into float32r SBUF tiles (full-rate PE fp32 mode).
  - Per 512-column batch chunk: PE-transpose x into xT, then per member:
    L1/L2 as psum[128,512] += W-tile^T @ actT-chunk, fused relu(psum+bias)
    alternating between the scalar and vector engines; L3 with swapped
    operands -> psum[128 batch, OUT] directly batch-major, bias added on
    the vector engine, DMA straight out.
"""

import os

import numpy as np

B, IN, H, OUT, E = 16384, 256, 512, 32, 8
GE, GB = 2, 4  # ensemble groups x batch shards = 8 cores
EG, BC = E // GE, B // GB  # 4 members, 4096 batch rows per core
P = 128
NB = 512  # batch chunk = matmul moving dim
NCHUNK = BC // NB
KI1, KI2 = IN // P, H // P  # contraction chunks for L1 (2) and L2/L3 (4)
M1 = H // P  # output-feature tiles for L1/L2 (4)

_NC_CACHE = {}


def _build():
    import concourse.mybir as mybir
    import concourse.tile as tile
    from concourse import bacc

    F32 = mybir.dt.float32
    F32R = mybir.dt.float32r
    Relu = mybir.ActivationFunctionType.Relu
    Alu = mybir.AluOpType

    nc = bacc.Bacc(
        "TRN2", target_bir_lowering=False, debug=False, num_devices=GE * GB
    )
    # x arrives pre-transposed (feature-major) from the host-side sharding.
    xtd = nc.dram_tensor("xt", [IN, BC], F32R, kind="ExternalInput")
    W1d = nc.dram_tensor("W1", [EG, IN, H], F32R, kind="ExternalInput")
    b1d = nc.dram_tensor("b1", [EG, H], F32, kind="ExternalInput")
    W2d = nc.dram_tensor("W2", [EG, H, H], F32R, kind="ExternalInput")
    b2d = nc.dram_tensor("b2", [EG, H], F32, kind="ExternalInput")
    W3d = nc.dram_tensor("W3", [EG, H, OUT], F32R, kind="ExternalInput")
    b3d = nc.dram_tensor("b3", [EG, OUT], F32, kind="ExternalInput")
    outd = nc.dram_tensor("out", [BC, EG, OUT], F32, kind="ExternalOutput")

    xtv = xtd.rearrange("(ki p) b -> p ki b", p=P)

    with tile.TileContext(nc) as tc:
        with (
            tc.tile_pool(name="weights", bufs=1) as wpool,
            tc.tile_pool(name="h", bufs=3) as hpool,
            tc.tile_pool(name="osb", bufs=8) as opool,
            tc.tile_pool(name="psum3", bufs=1, space="PSUM") as p3pool,
            tc.tile_pool(name="psum_mm", bufs=7, space="PSUM") as pbig,
        ):
            xTs = [
                wpool.tile([P, KI1, NB], F32R, tag=f"xT_{c}", name=f"xT_{c}")
                for c in range(NCHUNK)
            ]

            def load_xt(c, ksplit=False):
                # ksplit loads the k=0 half first so the k-phased first chunk
                # can start computing before the rest of the data lands.
                if ksplit:
                    for k in range(KI1):
                        nc.sync.dma_start(
                            xTs[c][:, k], xtv[:, k, c * NB : (c + 1) * NB]
                        )
                else:
                    nc.sync.dma_start(xTs[c], xtv[:, :, c * NB : (c + 1) * NB])

            # First-needed data first on the DMA queues: chunk-0 x tile.
            load_xt(0, ksplit=True)

            # Resident weights/biases in compute order (e-major), interleaved
            # with the next few x chunks so neither starves the other.
            W1sb, W2sb, W3sb, b1sb, b2sb, b3sb = [], [], [], [], [], []
            for e in range(EG):
                w1 = wpool.tile([P, KI1, H], F32R, tag=f"W1_{e}")
                w1v = W1d[e].rearrange("(ko p) m -> p ko m", p=P)
                if e == 0:
                    for k in range(KI1):
                        nc.sync.dma_start(w1[:, k], w1v[:, k])
                else:
                    nc.sync.dma_start(w1, w1v)
                W1sb.append(w1)
                t1 = wpool.tile([P, M1], F32, tag=f"b1_{e}")
                nc.sync.dma_start(t1, b1d[e].rearrange("(mo p) -> p mo", p=P))
                b1sb.append(t1)
                w2 = wpool.tile([P, KI2, H], F32R, tag=f"W2_{e}")
                nc.sync.dma_start(w2, W2d[e].rearrange("(ko p) m -> p ko m", p=P))
                W2sb.append(w2)
                t2 = wpool.tile([P, M1], F32, tag=f"b2_{e}")
                nc.sync.dma_start(t2, b2d[e].rearrange("(mo p) -> p mo", p=P))
                b2sb.append(t2)
                w3 = wpool.tile([P, KI2, OUT], F32R, tag=f"W3_{e}")
                nc.sync.dma_start(w3, W3d[e].rearrange("(ko p) o -> p ko o", p=P))
                W3sb.append(w3)
                t3 = wpool.tile([OUT, 1], F32, tag=f"b3_{e}")
                nc.sync.dma_start(t3, b3d[e][:, None])
                b3sb.append(t3)
                if e + 1 < NCHUNK:
                    load_xt(e + 1)

            for c in range(NCHUNK):
                bs = c * NB
                xT = xTs[c]
                if c >= 1 and c + EG < NCHUNK:
                    load_xt(c + EG)
                for e in range(EG):
                    h1 = hpool.tile([P, M1, NB], F32R, tag="h1")
                    # k-outer so the first chunk can start on the k=0 halves
                    # of xT/W1 while the k=1 halves are still in flight.
                    pss = [pbig.tile([P, NB], F32, tag="ps", name=f"ps_{m}") for m in range(M1)]
                    for k in range(KI1):
                        for m in range(M1):
                            nc.tensor.matmul(
                                pss[m],
                                W1sb[e][:, k, m * P : (m + 1) * P],
                                xT[:, k, :],
                                start=(k == 0),
                                stop=(k == KI1 - 1),
                            )
                    for m in range(M1):
                        ps = pss[m]
                        if m < 3:
                            nc.scalar.activation(
                                h1[:, m, :], ps, Relu, bias=b1sb[e][:, m : m + 1]
                            )
                        else:
                            nc.vector.tensor_scalar(
                                h1[:, m, :], ps,
                                b1sb[e][:, m : m + 1], 0.0,
                                Alu.add, Alu.max,
                            )
                    h2 = hpool.tile([P, KI2, NB], F32R, tag="h2")
                    for m in range(M1):
                        ps = pbig.tile([P, NB], F32, tag="ps")
                        for k in range(KI2):
                            nc.tensor.matmul(
                                ps,
                                W2sb[e][:, k, m * P : (m + 1) * P],
                                h1[:, k, :],
                                start=(k == 0),
                                stop=(k == KI2 - 1),
                            )
                        if m < 2:
                            nc.scalar.activation(
                                h2[:, m, :], ps, Relu, bias=b2sb[e][:, m : m + 1]
                            )
                        else:
                            nc.vector.tensor_scalar(
                                h2[:, m, :], ps,
                                b2sb[e][:, m : m + 1], 0.0,
                                Alu.add, Alu.max,
                            )
                    # L3 feature-major: psum[32, NB] = sum_k W3[k].T @ h2T[k].
                    # Tiny stationary (W3 chunk) keeps the PE weight port free.
                    ps3 = p3pool.tile([OUT, NB], F32, tag="ps3")
                    for k in range(KI2):
                        nc.tensor.matmul(
                            ps3,
                            W3sb[e][:, k, :],
                            h2[:, k, :],
                            start=(k == 0),
                            stop=(k == KI2 - 1),
                        )
                    o3 = opool.tile([OUT, NB], F32, tag="o3")
                    nc.vector.tensor_scalar_add(o3, ps3, b3sb[e])
                    # 32x32 block transpose: o3t[q, 32j+o] = o3[o, 32j+q],
                    # i.e. batch-major within each 32-wide block.
                    o3t = opool.tile([OUT, NB], F32, tag="o3t")
                    nc.vector.transpose(o3t, o3)
                    nc.sync.dma_start(
                        outd[bs : bs + NB]
                        .rearrange("(j q) eg o -> q j eg o", q=32)[:, :, e, :],
                        o3t.rearrange("q (j o) -> q j o", o=OUT),
                    )
    nc.compile()
    return nc


def _get_nc():
    if "nc" not in _NC_CACHE:
        _NC_CACHE["nc"] = _build()
    return _NC_CACHE["nc"]


LAST_RESULTS = None


def kernel(inputs, W1, b1, W2, b2, W3, b3):
    global LAST_RESULTS
    from concourse.bass_utils import run_bass_kernel_spmd

    nc = _get_nc()

    # Pre-transpose x to feature-major on the host (input marshaling, so the
    # device reads contraction-dim-on-partitions layout directly).
    XT = np.ascontiguousarray(np.asarray(inputs, dtype=np.float32).T)  # [IN, B]
    full = {
        "W1": np.asarray(W1, dtype=np.float32),
        "b1": np.asarray(b1, dtype=np.float32),
        "W2": np.asarray(W2, dtype=np.float32),
        "b2": np.asarray(b2, dtype=np.float32),
        "W3": np.asarray(W3, dtype=np.float32),
        "b3": np.asarray(b3, dtype=np.float32),
    }

    in_maps = []
    for c in range(GE * GB):
        ie, ib = divmod(c, GB)
        m = {"xt": np.ascontiguousarray(XT[:, ib * BC : (ib + 1) * BC])}
        for k, v in full.items():
            m[k] = np.ascontiguousarray(v[ie * EG : (ie + 1) * EG])
        in_maps.append(m)

    trace = bool(int(os.environ.get("KERNEL_TRACE", "0")))
    res = None
    for attempt in range(3):
        try:
            res = run_bass_kernel_spmd(
                nc, in_maps, core_ids=list(range(GE * GB)), trace=trace
            )
            break
        except Exception:
            # Occasional transient NRT_EXEC_UNIT_UNRECOVERABLE on the first
            # execution after a fresh compile; a retry on clean state passes.
            if attempt == 2:
                raise
    LAST_RESULTS = res

    out = np.empty((B, E, OUT), np.float32)
    for c in range(GE * GB):
        ie, ib = divmod(c, GB)
        out[ib * BC : (ib + 1) * BC, ie * EG : (ie + 1) * EG, :] = res.results[c][
            "out"
        ]
    return out


if __name__ == "__main__":
    rng = np.random.default_rng(0)
    ins = {
        "inputs": rng.standard_normal((B, IN), dtype=np.float32),
        "W1": rng.standard_normal((E, IN, H), dtype=np.float32) / np.sqrt(IN),
        "b1": np.zeros((E, H), np.float32),
        "W2": rng.standard_normal((E, H, H), dtype=np.float32) / np.sqrt(H),
        "b2": np.zeros((E, H), np.float32),
        "W3": rng.standard_normal((E, H, OUT), dtype=np.float32) / np.sqrt(H),
        "b3": np.zeros((E, OUT), np.float32),
    }
    got = kernel(**ins)
    h = np.maximum(np.einsum("bi,eih->ebh", ins["inputs"][:256], ins["W1"]), 0)
    h = np.maximum(np.einsum("ebh,ehk->ebk", h, ins["W2"]), 0)
    want = np.einsum("ebh,eho->beo", h, ins["W3"])
    err = np.abs(got[:256] - want).max() / np.abs(want).max()
    print("smoke rel err:", err)


# revision 31
# speedup vs baseline: 1.0471x; 1.0471x over previous
"""Trainium2 Bass kernel for an ensemble Q-network MLP.

Reference computation (per ensemble member e):
    h1 = relu(x @ W1[e] + b1[e])      x: [B, IN]
    h2 = relu(h1 @ W2[e] + b2[e])
    out[:, e, :] = h2 @ W3[e] + b3[e]  -> [B, E, OUT]

Shapes: B=16384, IN=256, H=512, OUT=32, E=8, fp32.

Sharding across 8 NeuronCores: 2 ensemble groups x 4 batch shards.
Each core processes EG=4 members on BC=4096 batch rows (equal FLOPs per
core; minimizes total DMA volume per core).

On-core plan (activations kept feature-major [feature_part, batch_free] so
the contraction dim always lands on SBUF partitions):
  - Weights DMA straight into float32r SBUF tiles (full-rate PE fp32 mode).
  - Per 512-column batch chunk: PE-transpose x into xT, then per member:
    L1/L2 as psum[128,512] += W-tile^T @ actT-chunk, fused relu(psum+bias)
    alternating between the scalar and vector engines; L3 with swapped
    operands -> psum[128 batch, OUT] directly batch-major, bias added on
    the vector engine, DMA straight out.
"""

import os

import numpy as np

B, IN, H, OUT, E = 16384, 256, 512, 32, 8
GE, GB = 2, 4  # ensemble groups x batch shards = 8 cores
EG, BC = E // GE, B // GB  # 4 members, 4096 batch rows per core
P = 128
NB = 512  # batch chunk = matmul moving dim
NCHUNK = BC // NB
KI1, KI2 = IN // P, H // P  # contraction chunks for L1 (2) and L2/L3 (4)
M1 = H // P  # output-feature tiles for L1/L2 (4)

_NC_CACHE = {}


def _build():
    import concourse.mybir as mybir
    import concourse.tile as tile
    from concourse import bacc

    F32 = mybir.dt.float32
    F32R = mybir.dt.float32r
    Relu = mybir.ActivationFunctionType.Relu
    Alu = mybir.AluOpType

    nc = bacc.Bacc(
        "TRN2", target_bir_lowering=False, debug=False, num_devices=GE * GB
    )
    # x arrives pre-transposed (feature-major) from the host-side sharding.
    xtd = nc.dram_tensor("xt", [IN, BC], F32R, kind="ExternalInput")
    W1d = nc.dram_tensor("W1", [EG, IN, H], F32R, kind="ExternalInput")
    b1d = nc.dram_tensor("b1", [EG, H], F32, kind="ExternalInput")
    W2d = nc.dram_tensor("W2", [EG, H, H], F32R, kind="ExternalInput")
    b2d = nc.dram_tensor("b2", [EG, H], F32, kind="ExternalInput")
    W3d = nc.dram_tensor("W3", [EG, H, OUT], F32R, kind="ExternalInput")
    b3d = nc.dram_tensor("b3", [EG, OUT], F32, kind="ExternalInput")
    outd = nc.dram_tensor("out", [BC, EG, OUT], F32, kind="ExternalOutput")

    xtv = xtd.rearrange("(ki p) b -> p ki b", p=P)

    with tile.TileContext(nc) as tc:
        with (
            tc.tile_pool(name="weights", bufs=1) as wpool,
            tc.tile_pool(name="h", bufs=3) as hpool,
            tc.tile_pool(name="osb", bufs=8) as opool,
            tc.tile_pool(name="psum3", bufs=1, space="PSUM") as p3pool,
            tc.tile_pool(name="psum_mm", bufs=7, space="PSUM") as pbig,
        ):
            xTs = [
                wpool.tile([P, KI1, NB], F32R, tag=f"xT_{c}", name=f"xT_{c}")
                for c in range(NCHUNK)
            ]

            def load_xt(c, ksplit=False):
                # ksplit loads the k=0 half first so the k-phased first chunk
                # can start computing before the rest of the data lands.
                if ksplit:
                    for k in range(KI1):
                        nc.sync.dma_start(
                            xTs[c][:, k], xtv[:, k, c * NB : (c + 1) * NB]
                        )
                else:
                    nc.sync.dma_start(xTs[c], xtv[:, :, c * NB : (c + 1) * NB])

            # First-needed data first on the DMA queues: chunk-0 x tile.
            load_xt(0, ksplit=True)

            # Resident weights/biases in compute order (e-major), interleaved
            # with the next few x chunks so neither starves the other.
            W1sb, W2sb, W3sb, b1sb, b2sb, b3sb = [], [], [], [], [], []
            for e in range(EG):
                w1 = wpool.tile([P, KI1, H], F32R, tag=f"W1_{e}")
                w1v = W1d[e].rearrange("(ko p) m -> p ko m", p=P)
                if e == 0:
                    for k in range(KI1):
                        nc.sync.dma_start(w1[:, k], w1v[:, k])
                else:
                    nc.sync.dma_start(w1, w1v)
                W1sb.append(w1)
                t1 = wpool.tile([P, M1], F32, tag=f"b1_{e}")
                nc.sync.dma_start(t1, b1d[e].rearrange("(mo p) -> p mo", p=P))
                b1sb.append(t1)
                w2 = wpool.tile([P, KI2, H], F32R, tag=f"W2_{e}")
                nc.sync.dma_start(w2, W2d[e].rearrange("(ko p) m -> p ko m", p=P))
                W2sb.append(w2)
                t2 = wpool.tile([P, M1], F32, tag=f"b2_{e}")
                nc.sync.dma_start(t2, b2d[e].rearrange("(mo p) -> p mo", p=P))
                b2sb.append(t2)
                w3 = wpool.tile([P, KI2, OUT], F32R, tag=f"W3_{e}")
                nc.sync.dma_start(w3, W3d[e].rearrange("(ko p) o -> p ko o", p=P))
                W3sb.append(w3)
                t3 = wpool.tile([OUT, 1], F32, tag=f"b3_{e}")
                nc.sync.dma_start(t3, b3d[e][:, None])
                b3sb.append(t3)
                if e + 1 < NCHUNK:
                    load_xt(e + 1)

            for c in range(NCHUNK):
                bs = c * NB
                xT = xTs[c]
                if c >= 1 and c + EG < NCHUNK:
                    load_xt(c + EG)
                for e in range(EG):
                    h1 = hpool.tile([P, M1, NB], F32R, tag="h1")
                    for m in range(M1):
                        ps = pbig.tile([P, NB], F32, tag="ps")
                        for k in range(KI1):
                            nc.tensor.matmul(
                                ps,
                                W1sb[e][:, k, m * P : (m + 1) * P],
                                xT[:, k, :],
                                start=(k == 0),
                                stop=(k == KI1 - 1),
                            )
                        if m < 3:
                            nc.scalar.activation(
                                h1[:, m, :], ps, Relu, bias=b1sb[e][:, m : m + 1]
                            )
                        else:
                            nc.vector.tensor_scalar(
                                h1[:, m, :], ps,
                                b1sb[e][:, m : m + 1], 0.0,
                                Alu.add, Alu.max,
                            )
                    h2 = hpool.tile([P, KI2, NB], F32R, tag="h2")
                    for m in range(M1):
                        ps = pbig.tile([P, NB], F32, tag="ps")
                        for k in range(KI2):
                            nc.tensor.matmul(
                                ps,
                                W2sb[e][:, k, m * P : (m + 1) * P],
                                h1[:, k, :],
                                start=(k == 0),
                                stop=(k == KI2 - 1),
                            )
                        if m < 2:
                            nc.scalar.activation(
                                h2[:, m, :], ps, Relu, bias=b2sb[e][:, m : m + 1]
                            )
                        else:
                            nc.vector.tensor_scalar(
                                h2[:, m, :], ps,
                                b2sb[e][:, m : m + 1], 0.0,
                                Alu.add, Alu.max,
                            )
                    # L3 feature-major: psum[32, NB] = sum_k W3[k].T @ h2T[k].
                    # Tiny stationary (W3 chunk) keeps the PE weight port free.
                    ps3 = p3pool.tile([OUT, NB], F32, tag="ps3")
                    for k in range(KI2):
                        nc.tensor.matmul(
                            ps3,
                            W3sb[e][:, k, :],
                            h2[:, k, :],
                            start=(k == 0),
                            stop=(k == KI2 - 1),
                        )
                    o3 = opool.tile([OUT, NB], F32, tag="o3")
                    nc.vector.tensor_scalar_add(o3, ps3, b3sb[e])
                    # 32x32 block transpose: o3t[q, 32j+o] = o3[o, 32j+q],
                    # i.e. batch-major within each 32-wide block.
                    o3t = opool.tile([OUT, NB], F32, tag="o3t")
                    nc.vector.transpose(o3t, o3)
                    nc.sync.dma_start(
                        outd[bs : bs + NB]
                        .rearrange("(j q) eg o -> q j eg o", q=32)[:, :, e, :],
                        o3t.rearrange("q (j o) -> q j o", o=OUT),
                    )
    nc.compile()
    return nc


def _get_nc():
    if "nc" not in _NC_CACHE:
        _NC_CACHE["nc"] = _build()
    return _NC_CACHE["nc"]


LAST_RESULTS = None


def kernel(inputs, W1, b1, W2, b2, W3, b3):
    global LAST_RESULTS
    from concourse.bass_utils import run_bass_kernel_spmd

    nc = _get_nc()

    # Pre-transpose x to feature-major on the host (input marshaling, so the
    # device reads contraction-dim-on-partitions layout directly).
    XT = np.ascontiguousarray(np.asarray(inputs, dtype=np.float32).T)  # [IN, B]
    full = {
        "W1": np.asarray(W1, dtype=np.float32),
        "b1": np.asarray(b1, dtype=np.float32),
        "W2": np.asarray(W2, dtype=np.float32),
        "b2": np.asarray(b2, dtype=np.float32),
        "W3": np.asarray(W3, dtype=np.float32),
        "b3": np.asarray(b3, dtype=np.float32),
    }

    in_maps = []
    for c in range(GE * GB):
        ie, ib = divmod(c, GB)
        m = {"xt": np.ascontiguousarray(XT[:, ib * BC : (ib + 1) * BC])}
        for k, v in full.items():
            m[k] = np.ascontiguousarray(v[ie * EG : (ie + 1) * EG])
        in_maps.append(m)

    trace = bool(int(os.environ.get("KERNEL_TRACE", "0")))
    res = None
    for attempt in range(3):
        try:
            res = run_bass_kernel_spmd(
                nc, in_maps, core_ids=list(range(GE * GB)), trace=trace
            )
            break
        except Exception:
            # Occasional transient NRT_EXEC_UNIT_UNRECOVERABLE on the first
            # execution after a fresh compile; a retry on clean state passes.
            if attempt == 2:
                raise
    LAST_RESULTS = res

    out = np.empty((B, E, OUT), np.float32)
    for c in range(GE * GB):
        ie, ib = divmod(c, GB)
        out[ib * BC : (ib + 1) * BC, ie * EG : (ie + 1) * EG, :] = res.results[c][
            "out"
        ]
    return out


if __name__ == "__main__":
    rng = np.random.default_rng(0)
    ins = {
        "inputs": rng.standard_normal((B, IN), dtype=np.float32),
        "W1": rng.standard_normal((E, IN, H), dtype=np.float32) / np.sqrt(IN),
        "b1": np.zeros((E, H), np.float32),
        "W2": rng.standard_normal((E, H, H), dtype=np.float32) / np.sqrt(H),
        "b2": np.zeros((E, H), np.float32),
        "W3": rng.standard_normal((E, H, OUT), dtype=np.float32) / np.sqrt(H),
        "b3": np.zeros((E, OUT), np.float32),
    }
    got = kernel(**ins)
    h = np.maximum(np.einsum("bi,eih->ebh", ins["inputs"][:256], ins["W1"]), 0)
    h = np.maximum(np.einsum("ebh,ehk->ebk", h, ins["W2"]), 0)
    want = np.einsum("ebh,eho->beo", h, ins["W3"])
    err = np.abs(got[:256] - want).max() / np.abs(want).max()
    print("smoke rel err:", err)


# revision 34
# speedup vs baseline: 1.0604x; 1.0126x over previous
"""Trainium2 Bass kernel for an ensemble Q-network MLP.

Reference computation (per ensemble member e):
    h1 = relu(x @ W1[e] + b1[e])      x: [B, IN]
    h2 = relu(h1 @ W2[e] + b2[e])
    out[:, e, :] = h2 @ W3[e] + b3[e]  -> [B, E, OUT]

Shapes: B=16384, IN=256, H=512, OUT=32, E=8, fp32.

Sharding across 8 NeuronCores: 2 ensemble groups x 4 batch shards.
Each core processes EG=4 members on BC=4096 batch rows (equal FLOPs per
core; minimizes total DMA volume per core).

On-core plan (activations kept feature-major [feature_part, batch_free] so
the contraction dim always lands on SBUF partitions):
  - Weights DMA straight into float32r SBUF tiles (full-rate PE fp32 mode).
  - Per 512-column batch chunk: PE-transpose x into xT, then per member:
    L1/L2 as psum[128,512] += W-tile^T @ actT-chunk, fused relu(psum+bias)
    alternating between the scalar and vector engines; L3 with swapped
    operands -> psum[128 batch, OUT] directly batch-major, bias added on
    the vector engine, DMA straight out.
"""

import os

import numpy as np

B, IN, H, OUT, E = 16384, 256, 512, 32, 8
GE, GB = 2, 4  # ensemble groups x batch shards = 8 cores
EG, BC = E // GE, B // GB  # 4 members, 4096 batch rows per core
P = 128
NB = 512  # batch chunk = matmul moving dim
NCHUNK = BC // NB
KI1, KI2 = IN // P, H // P  # contraction chunks for L1 (2) and L2/L3 (4)
M1 = H // P  # output-feature tiles for L1/L2 (4)

_NC_CACHE = {}


def _build():
    import concourse.mybir as mybir
    import concourse.tile as tile
    from concourse import bacc

    F32 = mybir.dt.float32
    F32R = mybir.dt.float32r
    Relu = mybir.ActivationFunctionType.Relu
    Alu = mybir.AluOpType

    nc = bacc.Bacc(
        "TRN2", target_bir_lowering=False, debug=False, num_devices=GE * GB
    )
    # x arrives pre-transposed (feature-major) from the host-side sharding.
    xtd = nc.dram_tensor("xt", [IN, BC], F32R, kind="ExternalInput")
    W1d = nc.dram_tensor("W1", [EG, IN, H], F32R, kind="ExternalInput")
    b1d = nc.dram_tensor("b1", [EG, H], F32, kind="ExternalInput")
    W2d = nc.dram_tensor("W2", [EG, H, H], F32R, kind="ExternalInput")
    b2d = nc.dram_tensor("b2", [EG, H], F32, kind="ExternalInput")
    W3d = nc.dram_tensor("W3", [EG, H, OUT], F32R, kind="ExternalInput")
    b3d = nc.dram_tensor("b3", [EG, OUT], F32, kind="ExternalInput")
    outd = nc.dram_tensor("out", [BC, EG, OUT], F32, kind="ExternalOutput")

    xtv = xtd.rearrange("(ki p) b -> p ki b", p=P)

    with tile.TileContext(nc) as tc:
        with (
            tc.tile_pool(name="weights", bufs=1) as wpool,
            tc.tile_pool(name="h", bufs=4) as hpool,
            tc.tile_pool(name="osb", bufs=8) as opool,
            tc.tile_pool(name="psum3", bufs=1, space="PSUM") as p3pool,
            tc.tile_pool(name="psum_mm", bufs=7, space="PSUM") as pbig,
        ):
            xTs = [
                wpool.tile([P, KI1, NB], F32R, tag=f"xT_{c}", name=f"xT_{c}")
                for c in range(NCHUNK)
            ]

            def load_xt(c, ksplit=False):
                # ksplit loads the k=0 half first so the k-phased first chunk
                # can start computing before the rest of the data lands.
                if ksplit:
                    for k in range(KI1):
                        nc.sync.dma_start(
                            xTs[c][:, k], xtv[:, k, c * NB : (c + 1) * NB]
                        )
                else:
                    nc.sync.dma_start(xTs[c], xtv[:, :, c * NB : (c + 1) * NB])

            # First-needed data first on the DMA queues: the k=0 halves of
            # the chunk-0 x tile and of W1[e0], in consumption order.
            nc.sync.dma_start(xTs[0][:, 0], xtv[:, 0, 0:NB])
            w1_0 = wpool.tile([P, KI1, H], F32R, tag="W1_0", name="w1_0")
            w1v0 = W1d[0].rearrange("(ko p) m -> p ko m", p=P)
            nc.sync.dma_start(w1_0[:, 0], w1v0[:, 0])
            nc.sync.dma_start(xTs[0][:, 1], xtv[:, 1, 0:NB])
            nc.sync.dma_start(w1_0[:, 1], w1v0[:, 1])

            # Resident weights/biases in compute order (e-major), interleaved
            # with the next few x chunks so neither starves the other.
            W1sb, W2sb, W3sb, b1sb, b2sb, b3sb = [], [], [], [], [], []
            for e in range(EG):
                if e == 0:
                    w1 = w1_0  # already loading (startup-critical)
                else:
                    w1 = wpool.tile([P, KI1, H], F32R, tag=f"W1_{e}")
                    nc.sync.dma_start(w1, W1d[e].rearrange("(ko p) m -> p ko m", p=P))
                W1sb.append(w1)
                t1 = wpool.tile([P, M1], F32, tag=f"b1_{e}")
                nc.sync.dma_start(t1, b1d[e].rearrange("(mo p) -> p mo", p=P))
                b1sb.append(t1)
                w2 = wpool.tile([P, KI2, H], F32R, tag=f"W2_{e}")
                nc.sync.dma_start(w2, W2d[e].rearrange("(ko p) m -> p ko m", p=P))
                W2sb.append(w2)
                t2 = wpool.tile([P, M1], F32, tag=f"b2_{e}")
                nc.sync.dma_start(t2, b2d[e].rearrange("(mo p) -> p mo", p=P))
                b2sb.append(t2)
                w3 = wpool.tile([P, KI2, OUT], F32R, tag=f"W3_{e}")
                nc.sync.dma_start(w3, W3d[e].rearrange("(ko p) o -> p ko o", p=P))
                W3sb.append(w3)
                t3 = wpool.tile([OUT, 1], F32, tag=f"b3_{e}")
                nc.sync.dma_start(t3, b3d[e][:, None])
                b3sb.append(t3)
                if e + 1 < NCHUNK:
                    load_xt(e + 1)

            for c in range(NCHUNK):
                bs = c * NB
                xT = xTs[c]
                if c >= 1 and c + EG < NCHUNK:
                    load_xt(c + EG)
                for e in range(EG):
                    h1 = hpool.tile([P, M1, NB], F32R, tag="h1")
                    for m in range(M1):
                        ps = pbig.tile([P, NB], F32, tag="ps")
                        for k in range(KI1):
                            nc.tensor.matmul(
                                ps,
                                W1sb[e][:, k, m * P : (m + 1) * P],
                                xT[:, k, :],
                                start=(k == 0),
                                stop=(k == KI1 - 1),
                            )
                        if m < 3:
                            nc.scalar.activation(
                                h1[:, m, :], ps, Relu, bias=b1sb[e][:, m : m + 1]
                            )
                        else:
                            nc.vector.tensor_scalar(
                                h1[:, m, :], ps,
                                b1sb[e][:, m : m + 1], 0.0,
                                Alu.add, Alu.max,
                            )
                    h2 = hpool.tile([P, KI2, NB], F32R, tag="h2")
                    for m in range(M1):
                        ps = pbig.tile([P, NB], F32, tag="ps")
                        for k in range(KI2):
                            nc.tensor.matmul(
                                ps,
                                W2sb[e][:, k, m * P : (m + 1) * P],
                                h1[:, k, :],
                                start=(k == 0),
                                stop=(k == KI2 - 1),
                            )
                        if m < 2:
                            nc.scalar.activation(
                                h2[:, m, :], ps, Relu, bias=b2sb[e][:, m : m + 1]
                            )
                        else:
                            nc.vector.tensor_scalar(
                                h2[:, m, :], ps,
                                b2sb[e][:, m : m + 1], 0.0,
                                Alu.add, Alu.max,
                            )
                    # L3 feature-major: psum[32, NB] = sum_k W3[k].T @ h2T[k].
                    # Tiny stationary (W3 chunk) keeps the PE weight port free.
                    ps3 = p3pool.tile([OUT, NB], F32, tag="ps3")
                    for k in range(KI2):
                        nc.tensor.matmul(
                            ps3,
                            W3sb[e][:, k, :],
                            h2[:, k, :],
                            start=(k == 0),
                            stop=(k == KI2 - 1),
                        )
                    o3 = opool.tile([OUT, NB], F32, tag="o3")
                    nc.vector.tensor_scalar_add(o3, ps3, b3sb[e])
                    # 32x32 block transpose: o3t[q, 32j+o] = o3[o, 32j+q],
                    # i.e. batch-major within each 32-wide block.
                    o3t = opool.tile([OUT, NB], F32, tag="o3t")
                    nc.vector.transpose(o3t, o3)
                    nc.sync.dma_start(
                        outd[bs : bs + NB]
                        .rearrange("(j q) eg o -> q j eg o", q=32)[:, :, e, :],
                        o3t.rearrange("q (j o) -> q j o", o=OUT),
                    )
    nc.compile()
    return nc


def _get_nc():
    if "nc" not in _NC_CACHE:
        _NC_CACHE["nc"] = _build()
    return _NC_CACHE["nc"]


LAST_RESULTS = None


def kernel(inputs, W1, b1, W2, b2, W3, b3):
    global LAST_RESULTS
    from concourse.bass_utils import run_bass_kernel_spmd

    nc = _get_nc()

    # Pre-transpose x to feature-major on the host (input marshaling, so the
    # device reads contraction-dim-on-partitions layout directly).
    XT = np.ascontiguousarray(np.asarray(inputs, dtype=np.float32).T)  # [IN, B]
    full = {
        "W1": np.asarray(W1, dtype=np.float32),
        "b1": np.asarray(b1, dtype=np.float32),
        "W2": np.asarray(W2, dtype=np.float32),
        "b2": np.asarray(b2, dtype=np.float32),
        "W3": np.asarray(W3, dtype=np.float32),
        "b3": np.asarray(b3, dtype=np.float32),
    }

    in_maps = []
    for c in range(GE * GB):
        ie, ib = divmod(c, GB)
        m = {"xt": np.ascontiguousarray(XT[:, ib * BC : (ib + 1) * BC])}
        for k, v in full.items():
            m[k] = np.ascontiguousarray(v[ie * EG : (ie + 1) * EG])
        in_maps.append(m)

    trace = bool(int(os.environ.get("KERNEL_TRACE", "0")))
    res = None
    for attempt in range(3):
        try:
            res = run_bass_kernel_spmd(
                nc, in_maps, core_ids=list(range(GE * GB)), trace=trace
            )
            break
        except Exception:
            # Occasional transient NRT_EXEC_UNIT_UNRECOVERABLE on the first
            # execution after a fresh compile; a retry on clean state passes.
            if attempt == 2:
                raise
    LAST_RESULTS = res

    out = np.empty((B, E, OUT), np.float32)
    for c in range(GE * GB):
        ie, ib = divmod(c, GB)
        out[ib * BC : (ib + 1) * BC, ie * EG : (ie + 1) * EG, :] = res.results[c][
            "out"
        ]
    return out


if __name__ == "__main__":
    rng = np.random.default_rng(0)
    ins = {
        "inputs": rng.standard_normal((B, IN), dtype=np.float32),
        "W1": rng.standard_normal((E, IN, H), dtype=np.float32) / np.sqrt(IN),
        "b1": np.zeros((E, H), np.float32),
        "W2": rng.standard_normal((E, H, H), dtype=np.float32) / np.sqrt(H),
        "b2": np.zeros((E, H), np.float32),
        "W3": rng.standard_normal((E, H, OUT), dtype=np.float32) / np.sqrt(H),
        "b3": np.zeros((E, OUT), np.float32),
    }
    got = kernel(**ins)
    h = np.maximum(np.einsum("bi,eih->ebh", ins["inputs"][:256], ins["W1"]), 0)
    h = np.maximum(np.einsum("ebh,ehk->ebk", h, ins["W2"]), 0)
    want = np.einsum("ebh,eho->beo", h, ins["W3"])
    err = np.abs(got[:256] - want).max() / np.abs(want).max()
    print("smoke rel err:", err)
